# revision 45
# baseline (speedup 1.0000x reference)
"""Multi-head attention (B=4, S=2048, D=1024, H=16) on 8 trn2 NeuronCores.

Sharding: tensor-parallel over heads — 2 heads per core. Each core computes
qkv for its 128 channels (column-parallel), full attention for its 8
(batch, head) pairs, and a row-parallel slice of the output projection.
Host gathers the 8 partial projection outputs and sums them (+ b_proj).

v2 design (vs the f32r baseline; cost-model time 358us vs 406us, with
further HW-only wins the local model does not credit):
  * all PE matmul operands are bf16; x and the weights are cast to bf16
    on the host, halving their DMA and removing the on-device casts.
  * PE instruction count minimized (the PE sequencer's per-LDW+MM
    dispatch is nearly critical): scores/attn@V/qkv all use 512-wide
    moving operands -> 1600 matmuls/core vs ~2700 in the baseline.
  * scores for the core's two heads are emitted adjacent so their K=64
    matmuls land in different PE row groups (tile_position (0,0)/(64,0))
    and run concurrently on the 128x128 array on silicon.
  * software-pipelined attention: per iter, scores(i) are emitted first
    (feeding the ACT exp ladder ASAP), then attn@V(i-1) — whose exps
    have all been issued, so its matmuls never stall the in-order PE
    queue — then qkv of batch b+1 / proj of batch b-1 as filler. The PE
    stays >90% busy and never idles long enough for the HAM clock gate
    to drop it back to 1.2 GHz.
  * exp: one ACTIVATE per ki-tile over both heads ([128, 2, 512] PSUM
    span = 1024 elements), double-buffered PSUM, output directly bf16.
  * softmax normalize: DVE reciprocal of the ones-column row of attn@V,
    one gpsimd partition_broadcast per iter, DVE scalar_tensor_tensor.
  * PSUM banks (16KB/partition): ss 2x2 + mm(qkv/transpose/proj) 2x1 +
    po 2x1 = 8.

Device layout is feature-major ("transposed"):
  qT/kT: [feat(128 = 2 heads x 64), tokens] bf16 (SBUF, persistent)
  v    : token-major tiles [128 tok, head, 65] bf16; col 64 == 1.0 so the
         attn@V matmul also emits the softmax denominator row.
"""

import numpy as np

import concourse.bass as bass
import concourse.mybir as mybir
import concourse.tile as tile
from concourse import bacc
from concourse.bass_utils import run_bass_kernel_spmd
from concourse.masks import make_identity

F32 = mybir.dt.float32
BF16 = mybir.dt.bfloat16

N_CORES = 8


def build_core_program(B=4, S=2048, D=1024, H=16, QB=512, TB=512, reps=1,
                       bufs_ss=2, GKT=1, bufs_mm=2):
    """One core's program (SPMD: every core runs this on its own shard)."""
    # PSUM budget (8 banks of 2KB): ss [128,2,GKT,QB] f32 = 2*GKT banks x
    # bufs_ss, mm [128,TB] f32 = TB/512 banks x bufs_mm, po = 1 bank x 2.
    assert 2 * GKT * bufs_ss + (TB // 512) * bufs_mm + 2 <= 8
    HD = D // H                 # 64
    HPC = H // N_CORES          # heads per core = 2
    CPC = HPC * HD              # channels per core = 128
    T = B * S                   # tokens = 8192
    SCALE = float(HD) ** -0.5
    KT = 128                    # ki tile (contraction tile for attn@V)
    NKT = S // KT               # ki tiles per batch = 16
    NTT = T // KT               # token tiles total = 64
    VW = HD + 1                 # v tile width per head incl. ones col = 65
    NQB = S // QB               # q blocks per batch = 4
    NTB = S // TB               # token blocks per batch in qkv = 2
    KD = D // 128               # contraction tiles for qkv = 8
    KH = KD // 2

    nc = bacc.Bacc("TRN2", target_bir_lowering=False, debug=False,
                   num_devices=N_CORES)

    xT_d = nc.dram_tensor("xT", [D, T], BF16, kind="ExternalInput")
    wq_d = nc.dram_tensor("wq", [D, CPC], BF16, kind="ExternalInput")
    wk_d = nc.dram_tensor("wk", [D, CPC], BF16, kind="ExternalInput")
    wv_d = nc.dram_tensor("wv", [D, CPC], BF16, kind="ExternalInput")
    bq_d = nc.dram_tensor("bq", [CPC, 1], F32, kind="ExternalInput")
    bk_d = nc.dram_tensor("bk", [CPC, 1], F32, kind="ExternalInput")
    bv_d = nc.dram_tensor("bv", [CPC, 1], F32, kind="ExternalInput")
    wp_d = nc.dram_tensor("wp", [CPC, D], BF16, kind="ExternalInput")
    # y partials ship as bf16: host sums the 8 cores' partials in f64, so
    # the only cost is one bf16 rounding per partial (~0.4% in quadrature
    # vs the 2e-2 budget) — and HBM write traffic halves.
    y_d = nc.dram_tensor("y", [T, D], BF16, kind="ExternalOutput")

    with tile.TileContext(nc) as tc:
        with tc.tile_pool(name="const", bufs=1) as const, \
             tc.tile_pool(name="persist", bufs=1) as persist:
            xT_r = xT_d.ap().rearrange("(a p) t -> p a t", p=128)
            qT_s = persist.tile([128, T], BF16)   # [2*64 feat, tokens]
            kT_s = persist.tile([128, T], BF16)
            v_s = persist.tile([128, NTT, HPC, VW], BF16)

            ident = const.tile([128, 128], BF16)
            ones_bf = const.tile([128, 1], BF16)
            actwarm = const.tile([128, 1], F32)
            wq_s = const.tile([128, KD, 128], BF16)
            wk_s = const.tile([128, KD, 128], BF16)
            wv_s = const.tile([128, KD, 128], BF16)
            wp_s = const.tile([128, D], BF16)
            bq_s = const.tile([CPC, 1], F32)
            bk_s = const.tile([CPC, 1], F32)
            bv_s = const.tile([CPC, 1], F32)

            for rep in range(reps):
              with tc.tile_pool(name="xin", bufs=3) as xin, \
                   tc.tile_pool(name="vtmp", bufs=3) as vtmp, \
                   tc.tile_pool(name="pT", bufs=2) as p_pool, \
                   tc.tile_pool(name="ao", bufs=2) as ao_pool, \
                   tc.tile_pool(name="rc", bufs=4) as rc_pool, \
                   tc.tile_pool(name="rcb", bufs=4) as rcb_pool, \
                   tc.tile_pool(name="yout", bufs=4) as yout, \
                   tc.tile_pool(name="mm_ps", bufs=bufs_mm,
                                space="PSUM") as mm_ps, \
                   tc.tile_pool(name="ss_ps", bufs=bufs_ss,
                                space="PSUM") as ss_ps, \
                   tc.tile_pool(name="po_ps", bufs=2, space="PSUM") as po_ps:

                def emit_qkv_chunk(b, tb, xt0=None):
                    """qkv projections for token block tb of batch b."""
                    off = b * S + tb * TB
                    if xt0 is None:
                        xt0 = xin.tile([128, KD, TB], BF16,
                                       name=f"xt0_{rep}_{b}_{tb}", tag="xt0")
                        nc.sync.dma_start(xt0[:], xT_r[:, :, off:off + TB])
                    for which, w_s, b_s in (("q", wq_s, bq_s),
                                            ("k", wk_s, bk_s),
                                            ("v", wv_s, bv_s)):
                        ps = mm_ps.tile([128, TB], F32,
                                        name=f"ps_{which}{rep}_{b}_{tb}",
                                        tag="mm")
                        for kd in range(KD):
                            nc.tensor.matmul(ps[:], w_s[:, kd, :],
                                             xt0[:, kd, :],
                                             start=(kd == 0),
                                             stop=(kd == KD - 1))
                        lsl = slice(off, off + TB)
                        if which == "q":
                            nc.vector.tensor_scalar_add(qT_s[:, lsl], ps[:],
                                                        b_s[:])
                        elif which == "k":
                            nc.vector.tensor_scalar_add(kT_s[:, lsl], ps[:],
                                                        b_s[:])
                        else:
                            vt = vtmp.tile([128, TB], BF16,
                                           name=f"vt{rep}_{b}_{tb}", tag="vt")
                            nc.vector.tensor_scalar_add(vt[:], ps[:], b_s[:])
                            for j in range(TB // 128):
                                ti = (off // 128) + j
                                pt = mm_ps.tile([128, 128], BF16,
                                                name=f"pt{rep}_{ti}",
                                                tag="mm")
                                nc.tensor.transpose(
                                    pt[:], vt[:, j * 128:(j + 1) * 128],
                                    ident[:])
                                dst = v_s[:, ti, :, 0:HD]
                                src = pt[:].rearrange("p (h w) -> p h w",
                                                      h=HPC)
                                nc.vector.tensor_copy(dst, src)

                def emit_proj_chunk(b, tt):
                    """output projection for token tile tt of batch b."""
                    ao = ao_tiles[b]
                    lt = ao[:, tt * 128:(tt + 1) * 128]
                    yt = yout.tile([128, D], BF16,
                                   name=f"yt{rep}_{b}_{tt}", tag="yt")
                    for half in range(D // TB):
                        py = mm_ps.tile([128, TB], F32,
                                        name=f"py{rep}_{b}_{tt}_{half}",
                                        tag="mm")
                        nc.tensor.matmul(
                            py[:], lt, wp_s[:, half * TB:(half + 1) * TB],
                            start=True, stop=True)
                        # DVE, not gpsimd: walrus forbids GPSIMD PSUM access
                        nc.vector.tensor_copy(
                            yt[:, half * TB:(half + 1) * TB], py[:])
                    nc.sync.dma_start(
                        y_d.ap()[b * S + tt * 128: b * S + (tt + 1) * 128, :],
                        yt[:])

                def emit_scores(b, qb, qw=QB, qo=0, pT=None, glo=0,
                                ghi=None):
                    """scores+exp for (batch b, q block) -> pipeline state.

                    attn@V for this state is emitted one iteration later
                    (emit_attnv), when every exp feeding it has already
                    been issued — so its matmuls never stall the in-order
                    PE queue waiting on the ACT ladder. glo/ghi emit a
                    subrange of ki-tile groups (the first iteration is
                    split around the batch-0 qkv chunks so the exp ladder
                    starts as soon as the first keys exist).
                    """
                    q0 = b * S + qb * QB + qo
                    qsl = slice(q0, q0 + qw)
                    if pT is None:
                        pT = p_pool.tile([128, NKT // GKT, HPC, GKT, qw],
                                         BF16, name=f"pT{rep}_{b}_{qb}_{qo}",
                                         tag="pT")
                    for g in range(glo, NKT // GKT if ghi is None else ghi):
                        ss = ss_ps.tile([128, HPC, GKT, qw], F32,
                                        name=f"ss{rep}_{b}_{qb}_{qo}_{g}",
                                        tag="ss")
                        for j in range(GKT):
                            kt = g * GKT + j
                            ksl = slice(b * S + kt * KT,
                                        b * S + (kt + 1) * KT)
                            for h in range(HPC):
                                hs = slice(h * HD, (h + 1) * HD)
                                nc.tensor.matmul(ss[:, h, j, :],
                                                 kT_s[hs, ksl],
                                                 qT_s[hs, qsl],
                                                 start=True, stop=True)
                        nc.scalar.activation(
                            pT[:, g, :, :, :], ss[:],
                            mybir.ActivationFunctionType.Exp,
                            scale=SCALE)
                    return (pT, b, qb, qw, qo)

                def emit_attnv(state):
                    """attn@V + softmax normalize for a pipelined iter."""
                    if state is None:
                        return
                    pT, b, qb, qw, qo = state
                    pos = []
                    for h in range(HPC):
                        po = po_ps.tile([VW, qw], F32,
                                        name=f"po{rep}_{b}_{qb}_{qo}_{h}",
                                        tag="po")
                        for kt in range(NKT):
                            ti = b * NKT + kt
                            nc.tensor.matmul(po[:], v_s[:, ti, h, :],
                                             pT[:, kt // GKT, h,
                                                kt % GKT, :],
                                             start=(kt == 0),
                                             stop=(kt == NKT - 1))
                        pos.append(po)
                    rc = rc_pool.tile([1, HPC, qw], F32,
                                      name=f"rc{rep}_{b}_{qb}_{qo}", tag="rc")
                    for h in range(HPC):
                        nc.vector.reciprocal(rc[:, h, :], pos[h][HD:VW, :])
                    rcb = rcb_pool.tile([HD, HPC, qw], F32,
                                        name=f"rcb{rep}_{b}_{qb}_{qo}",
                                        tag="rcb")
                    nc.gpsimd.partition_broadcast(rcb[:], rc[:])
                    ao = ao_tiles[b]
                    a0 = qb * QB + qo
                    for h in range(HPC):
                        hs = slice(h * HD, (h + 1) * HD)
                        nc.vector.scalar_tensor_tensor(
                            ao[hs, a0:a0 + qw], pos[h][0:HD, :],
                            1.0, rcb[:, h, :],
                            op0=mybir.AluOpType.mult,
                            op1=mybir.AluOpType.mult)

                # ---- setup + batch-0 qkv ----
                # first x block DMA goes out before the weight DMAs so the
                # PE's first matmul isn't queued behind them; its matmuls
                # are emitted AFTER the weight loads land (the tile
                # framework tracks deps in emission order only).
                xt_first = xin.tile([128, KD, TB], BF16,
                                    name=f"xt0_{rep}_first", tag="xt0")
                if rep == 0:
                    # wq first (small) so the first q matmul only waits for
                    # it plus the first half of the x block
                    nc.sync.dma_start(
                        wq_s[:],
                        wq_d.ap().rearrange("(a p) m -> p a m", p=128))
                nc.sync.dma_start(xt_first[:, 0:KH, :], xT_r[:, 0:KH, 0:TB])
                nc.sync.dma_start(xt_first[:, KH:KD, :], xT_r[:, KH:KD, 0:TB])
                if rep == 0:
                    for w_d, w_s in ((wk_d, wk_s), (wv_d, wv_s)):
                        nc.sync.dma_start(
                            w_s[:],
                            w_d.ap().rearrange("(a p) m -> p a m", p=128))
                    nc.sync.dma_start(wp_s[:], wp_d.ap()[:, :])
                    nc.sync.dma_start(bq_s[:], bq_d.ap()[:, :])
                    nc.sync.dma_start(bk_s[:], bk_d.ap()[:, :])
                    nc.sync.dma_start(bv_s[:], bv_d.ap()[:, :])
                    make_identity(nc, ident[:])
                    nc.vector.memset(actwarm[:], 0.0)
                    # dummy exp: pulls the ~2.7us ACT_TABLE_LOAD for the
                    # exp table set into the qkv warmup (ScalarE idle)
                    # instead of the first score ladder's critical path
                    nc.scalar.activation(actwarm[:], actwarm[:],
                                         mybir.ActivationFunctionType.Exp)
                    nc.vector.memset(ones_bf[:], 1.0)
                    nc.vector.tensor_copy(
                        v_s[:, :, :, HD:VW],
                        ones_bf[:, 0:1].to_broadcast([128, NTT, HPC, 1]))
                # batch-0 qkv with iter (0,0)'s scores interleaved: the
                # exp ladder starts once half of batch 0's keys exist
                # instead of after all four chunks
                emit_qkv_chunk(0, 0, xt0=xt_first)
                emit_qkv_chunk(0, 1)
                KPC = (TB // KT) // GKT   # score groups covered per chunk
                pT00 = p_pool.tile([128, NKT // GKT, HPC, GKT, QB], BF16,
                                   name=f"pT{rep}_0_0_0", tag="pT")
                emit_scores(0, 0, pT=pT00, glo=0, ghi=2 * KPC)
                for tb in range(2, NTB):
                    emit_qkv_chunk(0, tb)
                    emit_scores(0, 0, pT=pT00, glo=tb * KPC,
                                ghi=(tb + 1) * KPC)
                if B > 1:
                    emit_qkv_chunk(1, 0)

                ao_tiles = {}
                pending = (pT00, 0, 0, QB, 0)
                for b in range(B):
                    ao_tiles[b] = ao_pool.tile([128, S], BF16,
                                               name=f"ao{rep}_{b}", tag="ao")
                    # per iter: scores first (feed the ACT ladder ASAP),
                    # then the previous iter's attn@V (all exps issued),
                    # then qkv of batch b+1 / proj of batch b-1 as filler
                    for qb in range(NQB):
                        if b == 0 and qb == 0:
                            continue  # emitted early, interleaved with qkv
                        state = emit_scores(b, qb)
                        emit_attnv(pending)
                        pending = state
                        if b + 1 < B:
                            for u in range(NTB // NQB):
                                emit_qkv_chunk(b + 1,
                                               qb * (NTB // NQB) + u)
                            if NTB % NQB and qb < NTB % NQB:
                                emit_qkv_chunk(b + 1,
                                               NQB * (NTB // NQB) + qb)
                        if b > 0:
                            npt = (S // 128) // NQB
                            for u in range(npt):
                                emit_proj_chunk(b - 1, qb * npt + u)
                        if b == B - 1 and qb > 1:
                            # last batch: its own proj for q blocks whose
                            # (pipelined) attn@V is already emitted
                            npt = (S // 128) // NQB
                            for u in range(npt):
                                emit_proj_chunk(b, (qb - 2) * npt + u)
                # tail: ready proj work goes ahead of the final attn@V so
                # the in-order PE queue isn't blocked by its exp pacing
                npt = (S // 128) // NQB
                for tt in range((NQB - 2) * npt, (NQB - 1) * npt):
                    emit_proj_chunk(B - 1, tt)
                emit_attnv(pending)
                for tt in range((NQB - 1) * npt, S // 128):
                    emit_proj_chunk(B - 1, tt)

    nc.compile()
    return nc


def shard_inputs(x, w_qkv, b_qkv, w_proj, B=4, S=2048, D=1024, H=16):
    """Host-side sharding: returns in_maps for the 8 cores.

    x and the weight matrices are pre-cast to bf16 (matmul operand
    precision on device) to halve their DMA traffic; biases stay fp32.
    """
    import ml_dtypes
    bf16 = ml_dtypes.bfloat16
    HD = D // H
    HPC = H // N_CORES
    CPC = HPC * HD
    T = B * S
    x = np.asarray(x, dtype=np.float32)
    w_qkv = np.asarray(w_qkv, dtype=np.float32)
    b_qkv = np.asarray(b_qkv, dtype=np.float32)
    w_proj = np.asarray(w_proj, dtype=np.float32)
    xT = np.ascontiguousarray(x.reshape(T, D).T.astype(bf16))
    in_maps = []
    for c in range(N_CORES):
        sl = slice(c * CPC, (c + 1) * CPC)
        in_maps.append({
            "xT": xT,
            "wq": np.ascontiguousarray(w_qkv[:, 0 * D:1 * D][:, sl].astype(bf16)),
            "wk": np.ascontiguousarray(w_qkv[:, 1 * D:2 * D][:, sl].astype(bf16)),
            "wv": np.ascontiguousarray(w_qkv[:, 2 * D:3 * D][:, sl].astype(bf16)),
            "bq": np.ascontiguousarray(b_qkv[0 * D:1 * D][sl]).reshape(CPC, 1),
            "bk": np.ascontiguousarray(b_qkv[1 * D:2 * D][sl]).reshape(CPC, 1),
            "bv": np.ascontiguousarray(b_qkv[2 * D:3 * D][sl]).reshape(CPC, 1),
            "wp": np.ascontiguousarray(w_proj[sl, :].astype(bf16)),
        })
    return in_maps


_NC_CACHE = {}


def _get_nc():
    if "nc" not in _NC_CACHE:
        _NC_CACHE["nc"] = build_core_program()
    return _NC_CACHE["nc"]


def kernel(x, w_qkv, b_qkv, w_proj, b_proj, _trace=False):
    B, S, D = 4, 2048, 1024
    nc = _get_nc()
    in_maps = shard_inputs(x, w_qkv, b_qkv, w_proj, B=B, S=S, D=D)
    res = run_bass_kernel_spmd(nc, in_maps, core_ids=list(range(N_CORES)),
                               trace=_trace)
    y = res.results[0]["y"].astype(np.float64)
    for i in range(1, N_CORES):
        y += res.results[i]["y"]
    y += np.asarray(b_proj, dtype=np.float64)
    out = y.astype(np.float32).reshape(B, S, D)
    if _trace:
        return out, res
    return out
